# revision 29
# baseline (speedup 1.0000x reference)
"""Pairwise squared Euclidean distance dist[i,j] = ||s_i - t_j||^2 on 8
Trainium2 NeuronCores.

Full inputs s [8192, 512] f32, t [8192, 512] f32 -> dist [8192, 8192] f32.

Strategy: dist = s_sq[:,None] + t_sq[None,:] - 2 s @ t^T.
2D shard over the 8 cores: 4 s-row blocks x 2 t-row blocks; each core
computes a [2048, 4096] output block.

Per-core kernel (v2 -- fp8 DoubleRow + fp16 out):
  - Inputs quantized to fp8e4 on host (measured end-to-end rel err 7.8e-3
    vs the 2e-2 gate; quantization dominates, GEMM accumulates in fp32).
  - cross = (-2 s_blk) @ t_blk^T on TensorE in fp8 DoubleRow perf mode:
    each matmul contracts K=256 (two 128-row k-tiles packed in the [K,2,*]
    AP), so a [128,512] psum group takes 2 matmuls.
  - Row norms are added two ways, splitting the epilogue across engines:
      * DVE tiles: one scalar_tensor_tensor (psum + ssq[p]) + tq[j].
      * ACT tiles: a K=2 f32r rank-1 matmul (ssq x ones + ones x tsq)
        accumulates the norms straight into PSUM, then ScalarE does a pure
        Copy psum -> fp16.  (ACT cannot add a free-dim vector itself.)
  - Output written as fp16 (halves the dominant DMA stream; exact f32
    upcast on host after gather).  All DMA contiguous runs >= 512 B.
"""
from contextlib import ExitStack

import numpy as np

import concourse.bacc as bacc
import concourse.tile as tile
from concourse import mybir
from concourse.bass_utils import run_bass_kernel_spmd

F32 = mybir.dt.float32
F32R = mybir.dt.float32r
F16 = mybir.dt.float16
F8 = mybir.dt.float8e4
BF16 = mybir.dt.bfloat16

N_S, N_T, D = 8192, 8192, 512      # full problem shape (hardcoded)
SB, TB = 4, 2                      # s-blocks x t-blocks = 8 cores
MS, NS = N_S // SB, N_T // TB      # per-core block: 2048 x 4096
KP = 2                             # k-pairs; each DoubleRow matmul eats 256
MT = MS // 128                     # 16 m-tiles
NH = 2                             # n output halves of 2048 cols
NQ = 2                             # 1024-col quarters per half
ACT_TILES = 35                     # of the 64 [128,1024] epilogue tiles
DR = mybir.MatmulPerfMode.DoubleRow

_CACHE = {}


def _build():
    nc = bacc.Bacc("TRN2", target_bir_lowering=False, debug=False, num_devices=8)
    sT_ap = nc.dram_tensor("sT8", [128, KP, 2, MS], F8, kind="ExternalInput").ap()
    tT_ap = nc.dram_tensor("tT8", [KP, 128, 2, NS], F8, kind="ExternalInput").ap()
    ssq_ap = nc.dram_tensor("ssq", [128, MT], F32, kind="ExternalInput").ap()
    r1_ap = nc.dram_tensor("r1", [2, MS + NS], F32R, kind="ExternalInput").ap()
    tsq_ap = nc.dram_tensor("tsq", [1, NS], F32, kind="ExternalInput").ap()
    out_ap = nc.dram_tensor("out", [MS, NS], F16, kind="ExternalOutput").ap()

    # epilogue-engine assignment for the 64 (nh, m, q) tiles: spread the
    # ACT_TILES ScalarE tiles evenly among the DVE ones
    def is_act(idx):
        return round((idx + 1) * ACT_TILES / 64) - round(idx * ACT_TILES / 64) == 1

    with tile.TileContext(nc) as tc, ExitStack() as ctx:
        w_pool = ctx.enter_context(tc.tile_pool(name="w", bufs=1))
        r_pool = ctx.enter_context(tc.tile_pool(name="r", bufs=2))
        q_pool = ctx.enter_context(tc.tile_pool(name="q", bufs=2))
        c_pool = ctx.enter_context(tc.tile_pool(name="c", bufs=1))
        ot_pool = ctx.enter_context(tc.tile_pool(name="ot", bufs=16))
        ps_pool = ctx.enter_context(tc.tile_pool(name="ps", bufs=4, space="PSUM"))

        # sT8 resident [128, KP, 2, MS] (both k-pairs in one tile, so the
        # streamed m-chunks are single big DMAs that beat the HWDGE cadence)
        sT_sb = w_pool.tile([128, KP, 2, MS], F8, tag="w", name="w")
        r_tiles = [[None] * KP for _ in range(NH)]
        tq_tiles = [None] * NH

        def load_half(h, chunked=False):
            csls = (
                [slice(h * 2048 + i * 1024, h * 2048 + (i + 1) * 1024) for i in range(2)]
                if chunked else [slice(h * 2048, (h + 1) * 2048)]
            )
            for kp in range(KP):
                r = r_pool.tile([128, 2, 2048], F8, tag=f"r{kp}", name=f"r{kp}")
                r_tiles[h][kp] = r
            for csl in csls:
                dsl = slice(csl.start - h * 2048, csl.stop - h * 2048)
                for kp in range(KP):
                    nc.sync.dma_start(
                        out=r_tiles[h][kp][:, :, dsl], in_=tT_ap[kp][:, :, csl]
                    )

        # PE warm-up: dummy bf16 matmuls on a zeroed scratch while the first
        # loads stream in, so the p-state ramp completes before real data.
        scratch = c_pool.tile([128, 512], BF16, tag="scratch", name="scratch")
        nc.vector.memset(scratch[:], 0.0)
        warm = ps_pool.tile([128, 1024], F32, tag="ps", name="warm")
        for _ in range(8):
            nc.tensor.matmul(
                warm[:, 0:512], lhsT=scratch[:, 0:128], rhs=scratch[:],
                start=True, stop=True,
            )

        with tc.high_priority():
            # One tiny const DMA first on the SP/HWDGE queue (~90ns transfer):
            # the first ACT epilogue needs r1m (rank-1 matmul), so it gates
            # the first out-DMA.  tr/ssq go via gpsimd/SWDGE, off the
            # HWDGE cadence.
            r1_sb = c_pool.tile([2, MS + NS], F32R, tag="r1", name="r1")
            nc.sync.dma_start(out=r1_sb[:], in_=r1_ap[:])
            tr = q_pool.tile([1, NS], F32, tag="tr", name="tr")
            nc.gpsimd.dma_start(out=tr[:], in_=tsq_ap[:])
            ssq_sb = c_pool.tile([128, MT], F32, tag="ssq", name="ssq")
            nc.gpsimd.dma_start(out=ssq_sb[:], in_=ssq_ap[:])
            for h in range(NH):
                tq = q_pool.tile([128, 2048], F32, tag="tq", name="tq")
                nc.gpsimd.partition_broadcast(
                    tq[:], tr[:, h * 2048:(h + 1) * 2048]
                )
                tq_tiles[h] = tq
            # first m-chunk of both k-pair weights, then the first t columns,
            # then the rest of the streams
            nc.sync.dma_start(
                out=sT_sb[:, :, :, 0:512], in_=sT_ap[:, :, :, 0:512]
            )
            load_half(0, chunked=True)
            nc.sync.dma_start(
                out=sT_sb[:, :, :, 512:MS], in_=sT_ap[:, :, :, 512:MS]
            )
            load_half(1)

        tile_idx = 0
        for h in range(NH):
            r_sb = r_tiles[h]
            tq = tq_tiles[h]
            for m in range(MT):
                msl = slice(m * 128, (m + 1) * 128)
                ot = ot_pool.tile([128, 2048], F16, tag="ot", name="ot")
                for q in range(NQ):
                    act = is_act(tile_idx)
                    tile_idx += 1
                    ps = ps_pool.tile([128, 1024], F32, tag="ps", name="ps")
                    for g in range(2):          # 512-col psum groups
                        gsl = slice(q * 1024 + g * 512, q * 1024 + (g + 1) * 512)
                        psl = slice(g * 512, (g + 1) * 512)
                        for kp in range(KP):
                            nc.tensor.matmul(
                                ps[:, psl],
                                lhsT=sT_sb[:, kp, :, msl],
                                rhs=r_sb[kp][:, :, gsl],
                                start=(kp == 0),
                                stop=(kp == KP - 1) and not act,
                                perf_mode=DR,
                            )
                        if act:
                            # rank-1 norms: psum += ssq[m] x 1 + 1 x tsq
                            nc.tensor.matmul(
                                ps[:, psl],
                                lhsT=r1_sb[:, msl],
                                rhs=r1_sb[:, MS + h * 2048 + q * 1024 + g * 512:
                                          MS + h * 2048 + q * 1024 + (g + 1) * 512],
                                start=False,
                                stop=True,
                            )
                    osl = slice(q * 1024, (q + 1) * 1024)
                    if act:
                        nc.scalar.activation(
                            ot[:, osl], ps[:],
                            mybir.ActivationFunctionType.Copy,
                        )
                    else:
                        nc.vector.scalar_tensor_tensor(
                            ot[:, osl],
                            ps[:],
                            ssq_sb[:, m:m + 1],
                            tq[:, osl],
                            op0=mybir.AluOpType.add,
                            op1=mybir.AluOpType.add,
                        )
                    if h == NH - 1 and m >= MT - 2:
                        # tail: fire each quarter as soon as it's ready so
                        # the final DMA chain after the last matmul is short
                        nc.sync.dma_start(
                            out=out_ap[msl, h * 2048 + q * 1024:
                                       h * 2048 + (q + 1) * 1024],
                            in_=ot[:, osl],
                        )
                if not (h == NH - 1 and m >= MT - 2):
                    nc.sync.dma_start(
                        out=out_ap[msl, h * 2048:(h + 1) * 2048],
                        in_=ot[:],
                    )
    nc.compile()
    return nc


def _prep_in_maps(s: np.ndarray, t: np.ndarray) -> list[dict[str, np.ndarray]]:
    import ml_dtypes

    f8 = ml_dtypes.float8_e4m3
    ssq_full = np.einsum("ij,ij->i", s.astype(np.float64), s.astype(np.float64))
    tsq_full = np.einsum("ij,ij->i", t.astype(np.float64), t.astype(np.float64))
    in_maps = []
    for c in range(8):
        si, tj = c // TB, c % TB
        s_blk = s[si * MS:(si + 1) * MS]
        t_blk = t[tj * NS:(tj + 1) * NS]
        # [k, kp, i, m] with source row = 256*kp + 128*i + k
        sT8 = np.ascontiguousarray(
            (-2.0 * s_blk).T.reshape(KP, 2, 128, MS).transpose(2, 0, 1, 3)
        ).astype(f8)
        tT8 = np.ascontiguousarray(
            t_blk.T.reshape(KP, 2, 128, NS).transpose(0, 2, 1, 3)
        ).astype(f8)
        ssq = ssq_full[si * MS:(si + 1) * MS].astype(np.float32)
        tsq = tsq_full[tj * NS:(tj + 1) * NS].astype(np.float32)
        # r1[:, 0:MS] = rank-1 lhsT (row0 ssq, row1 ones);
        # r1[:, MS:]  = rank-1 rhs  (row0 ones, row1 tsq)
        r1 = np.empty((2, MS + NS), np.float32)
        r1[0, :MS] = ssq
        r1[1, :MS] = 1.0
        r1[0, MS:] = 1.0
        r1[1, MS:] = tsq
        in_maps.append({
            "sT8": sT8,
            "tT8": tT8,
            "ssq": np.ascontiguousarray(ssq.reshape(MT, 128).T),
            "r1": r1,
            "tsq": np.ascontiguousarray(tsq.reshape(1, NS)),
        })
    return in_maps


def _run(s: np.ndarray, t: np.ndarray, trace: bool = False, tmpdir=None):
    if "nc" not in _CACHE:
        _CACHE["nc"] = _build()
    nc = _CACHE["nc"]
    in_maps = _prep_in_maps(s, t)
    res = run_bass_kernel_spmd(
        nc, in_maps, core_ids=list(range(8)), trace=trace, tmpdir=tmpdir
    )
    out = np.empty((N_S, N_T), dtype=np.float32)
    for c in range(8):
        si, tj = c // TB, c % TB
        out[si * MS:(si + 1) * MS, tj * NS:(tj + 1) * NS] = (
            res.results[c]["out"].astype(np.float32)
        )
    return out, res


def kernel(s: np.ndarray, t: np.ndarray) -> np.ndarray:
    s = np.ascontiguousarray(np.asarray(s, dtype=np.float32))
    t = np.ascontiguousarray(np.asarray(t, dtype=np.float32))
    assert s.shape == (N_S, D) and t.shape == (N_T, D)
    out, _ = _run(s, t)
    return out


# revision 32
# speedup vs baseline: 1.0079x; 1.0079x over previous
"""Pairwise squared Euclidean distance dist[i,j] = ||s_i - t_j||^2 on 8
Trainium2 NeuronCores.

Full inputs s [8192, 512] f32, t [8192, 512] f32 -> dist [8192, 8192] f32.

Strategy: dist = s_sq[:,None] + t_sq[None,:] - 2 s @ t^T.
2D shard over the 8 cores: 4 s-row blocks x 2 t-row blocks; each core
computes a [2048, 4096] output block.

Per-core kernel (v2 -- fp8 DoubleRow + fp16 out):
  - Inputs quantized to fp8e4 on host (measured end-to-end rel err 7.8e-3
    vs the 2e-2 gate; quantization dominates, GEMM accumulates in fp32).
  - cross = (-2 s_blk) @ t_blk^T on TensorE in fp8 DoubleRow perf mode:
    each matmul contracts K=256 (two 128-row k-tiles packed in the [K,2,*]
    AP), so a [128,512] psum group takes 2 matmuls.
  - Row norms are added two ways, splitting the epilogue across engines:
      * DVE tiles: one scalar_tensor_tensor (psum + ssq[p]) + tq[j].
      * ACT tiles: a K=2 f32r rank-1 matmul (ssq x ones + ones x tsq)
        accumulates the norms straight into PSUM, then ScalarE does a pure
        Copy psum -> fp16.  (ACT cannot add a free-dim vector itself.)
  - Output written as fp16 (halves the dominant DMA stream; exact f32
    upcast on host after gather).  All DMA contiguous runs >= 512 B.
"""
from contextlib import ExitStack

import numpy as np

import concourse.bacc as bacc
import concourse.tile as tile
from concourse import mybir
from concourse.bass_utils import run_bass_kernel_spmd

F32 = mybir.dt.float32
F32R = mybir.dt.float32r
F16 = mybir.dt.float16
F8 = mybir.dt.float8e4
BF16 = mybir.dt.bfloat16

N_S, N_T, D = 8192, 8192, 512      # full problem shape (hardcoded)
SB, TB = 4, 2                      # s-blocks x t-blocks = 8 cores
MS, NS = N_S // SB, N_T // TB      # per-core block: 2048 x 4096
KP = 2                             # k-pairs; each DoubleRow matmul eats 256
MT = MS // 128                     # 16 m-tiles
NH = 2                             # n output halves of 2048 cols
NQ = 2                             # 1024-col quarters per half
ACT_TILES = 35                     # of the 64 [128,1024] epilogue tiles
DR = mybir.MatmulPerfMode.DoubleRow

_CACHE = {}


def _build():
    nc = bacc.Bacc("TRN2", target_bir_lowering=False, debug=False, num_devices=8)
    sT_ap = nc.dram_tensor("sT8", [128, KP, 2, MS], F8, kind="ExternalInput").ap()
    tT_ap = nc.dram_tensor("tT8", [KP, 128, 2, NS], F8, kind="ExternalInput").ap()
    ssq_ap = nc.dram_tensor("ssq", [128, MT], F32, kind="ExternalInput").ap()
    r1_ap = nc.dram_tensor("r1", [2, MS + NS], F32R, kind="ExternalInput").ap()
    tsq_ap = nc.dram_tensor("tsq", [1, NS], F32, kind="ExternalInput").ap()
    out_ap = nc.dram_tensor("out", [MS, NS], F16, kind="ExternalOutput").ap()

    # epilogue-engine assignment for the 64 (nh, m, q) tiles: spread the
    # ACT_TILES ScalarE tiles evenly among the DVE ones
    def is_act(idx):
        return round((idx + 1) * ACT_TILES / 64) - round(idx * ACT_TILES / 64) == 1

    with tile.TileContext(nc) as tc, ExitStack() as ctx:
        w_pool = ctx.enter_context(tc.tile_pool(name="w", bufs=1))
        r_pool = ctx.enter_context(tc.tile_pool(name="r", bufs=2))
        q_pool = ctx.enter_context(tc.tile_pool(name="q", bufs=2))
        c_pool = ctx.enter_context(tc.tile_pool(name="c", bufs=1))
        ot_pool = ctx.enter_context(tc.tile_pool(name="ot", bufs=16))
        ps_pool = ctx.enter_context(tc.tile_pool(name="ps", bufs=4, space="PSUM"))

        # sT8 resident [128, KP, 2, MS] (both k-pairs in one tile, so the
        # streamed m-chunks are single big DMAs that beat the HWDGE cadence)
        sT_sb = w_pool.tile([128, KP, 2, MS], F8, tag="w", name="w")
        r_tiles = [[None] * KP for _ in range(NH)]
        tq_tiles = [None] * NH

        def load_half(h, chunked=False, skip_first=False):
            csls = (
                [slice(h * 2048 + i * 1024, h * 2048 + (i + 1) * 1024) for i in range(2)]
                if chunked else [slice(h * 2048, (h + 1) * 2048)]
            )
            for kp in range(KP):
                if r_tiles[h][kp] is None:
                    r_tiles[h][kp] = r_pool.tile(
                        [128, 2, 2048], F8, tag=f"r{kp}", name=f"r{kp}"
                    )
            for ci, csl in enumerate(csls):
                dsl = slice(csl.start - h * 2048, csl.stop - h * 2048)
                for kp in range(KP):
                    if skip_first and ci == 0 and kp == 0:
                        continue  # already issued ahead of r1
                    nc.sync.dma_start(
                        out=r_tiles[h][kp][:, :, dsl], in_=tT_ap[kp][:, :, csl]
                    )

        # PE warm-up: dummy bf16 matmuls on a zeroed scratch while the first
        # loads stream in, so the p-state ramp completes before real data.
        scratch = c_pool.tile([128, 512], BF16, tag="scratch", name="scratch")
        nc.vector.memset(scratch[:], 0.0)
        warm = ps_pool.tile([128, 1024], F32, tag="ps", name="warm")
        for _ in range(8):
            nc.tensor.matmul(
                warm[:, 0:512], lhsT=scratch[:, 0:128], rhs=scratch[:],
                start=True, stop=True,
            )

        with tc.high_priority():
            # tr/ssq go via gpsimd/SWDGE, off the HWDGE cadence.
            tr = q_pool.tile([1, NS], F32, tag="tr", name="tr")
            nc.gpsimd.dma_start(out=tr[:], in_=tsq_ap[:])
            ssq_sb = c_pool.tile([128, MT], F32, tag="ssq", name="ssq")
            nc.gpsimd.dma_start(out=ssq_sb[:], in_=ssq_ap[:])
            for h in range(NH):
                tq = q_pool.tile([128, 2048], F32, tag="tq", name="tq")
                nc.gpsimd.partition_broadcast(
                    tq[:], tr[:, h * 2048:(h + 1) * 2048]
                )
                tq_tiles[h] = tq
            # Stream order on SP/HWDGE: two big loads first, then the tiny r1
            # const (~137ns transfer) -- at depth 3 the transfer backlog has
            # grown enough to absorb its 650ns HWDGE slot entirely, and its
            # sem (~4.5us) still beats the first ACT epilogue (~5.5us) that
            # needs it for the rank-1 matmul.  The t-halves load unchunked:
            # the 1456ns transfers keep every HWDGE slot covered.
            nc.sync.dma_start(
                out=sT_sb[:, :, :, 0:512], in_=sT_ap[:, :, :, 0:512]
            )
            r_sb0 = r_pool.tile([128, 2, 2048], F8, tag="r0", name="r0")
            r_tiles[0][0] = r_sb0
            nc.sync.dma_start(out=r_sb0[:], in_=tT_ap[0][:, :, 0:2048])
            r1_sb = c_pool.tile([2, MS + NS], F32R, tag="r1", name="r1")
            nc.sync.dma_start(out=r1_sb[:], in_=r1_ap[:])
            load_half(0, skip_first=True)
            nc.sync.dma_start(
                out=sT_sb[:, :, :, 512:MS], in_=sT_ap[:, :, :, 512:MS]
            )
            load_half(1)

        tile_idx = 0
        for h in range(NH):
            r_sb = r_tiles[h]
            tq = tq_tiles[h]
            for m in range(MT):
                msl = slice(m * 128, (m + 1) * 128)
                ot = ot_pool.tile([128, 2048], F16, tag="ot", name="ot")
                for q in range(NQ):
                    act = is_act(tile_idx)
                    tile_idx += 1
                    ps = ps_pool.tile([128, 1024], F32, tag="ps", name="ps")
                    for g in range(2):          # 512-col psum groups
                        gsl = slice(q * 1024 + g * 512, q * 1024 + (g + 1) * 512)
                        psl = slice(g * 512, (g + 1) * 512)
                        for kp in range(KP):
                            nc.tensor.matmul(
                                ps[:, psl],
                                lhsT=sT_sb[:, kp, :, msl],
                                rhs=r_sb[kp][:, :, gsl],
                                start=(kp == 0),
                                stop=(kp == KP - 1) and not act,
                                perf_mode=DR,
                            )
                        if act:
                            # rank-1 norms: psum += ssq[m] x 1 + 1 x tsq
                            nc.tensor.matmul(
                                ps[:, psl],
                                lhsT=r1_sb[:, msl],
                                rhs=r1_sb[:, MS + h * 2048 + q * 1024 + g * 512:
                                          MS + h * 2048 + q * 1024 + (g + 1) * 512],
                                start=False,
                                stop=True,
                            )
                    osl = slice(q * 1024, (q + 1) * 1024)
                    if act:
                        nc.scalar.activation(
                            ot[:, osl], ps[:],
                            mybir.ActivationFunctionType.Copy,
                        )
                    else:
                        nc.vector.scalar_tensor_tensor(
                            ot[:, osl],
                            ps[:],
                            ssq_sb[:, m:m + 1],
                            tq[:, osl],
                            op0=mybir.AluOpType.add,
                            op1=mybir.AluOpType.add,
                        )
                    if h == NH - 1 and m >= MT - 2:
                        # tail: fire each quarter as soon as it's ready so
                        # the final DMA chain after the last matmul is short
                        nc.sync.dma_start(
                            out=out_ap[msl, h * 2048 + q * 1024:
                                       h * 2048 + (q + 1) * 1024],
                            in_=ot[:, osl],
                        )
                if not (h == NH - 1 and m >= MT - 2):
                    nc.sync.dma_start(
                        out=out_ap[msl, h * 2048:(h + 1) * 2048],
                        in_=ot[:],
                    )
    nc.compile()
    return nc


def _prep_in_maps(s: np.ndarray, t: np.ndarray) -> list[dict[str, np.ndarray]]:
    import ml_dtypes

    f8 = ml_dtypes.float8_e4m3
    ssq_full = np.einsum("ij,ij->i", s.astype(np.float64), s.astype(np.float64))
    tsq_full = np.einsum("ij,ij->i", t.astype(np.float64), t.astype(np.float64))
    in_maps = []
    for c in range(8):
        si, tj = c // TB, c % TB
        s_blk = s[si * MS:(si + 1) * MS]
        t_blk = t[tj * NS:(tj + 1) * NS]
        # [k, kp, i, m] with source row = 256*kp + 128*i + k
        sT8 = np.ascontiguousarray(
            (-2.0 * s_blk).T.reshape(KP, 2, 128, MS).transpose(2, 0, 1, 3)
        ).astype(f8)
        tT8 = np.ascontiguousarray(
            t_blk.T.reshape(KP, 2, 128, NS).transpose(0, 2, 1, 3)
        ).astype(f8)
        ssq = ssq_full[si * MS:(si + 1) * MS].astype(np.float32)
        tsq = tsq_full[tj * NS:(tj + 1) * NS].astype(np.float32)
        # r1[:, 0:MS] = rank-1 lhsT (row0 ssq, row1 ones);
        # r1[:, MS:]  = rank-1 rhs  (row0 ones, row1 tsq)
        r1 = np.empty((2, MS + NS), np.float32)
        r1[0, :MS] = ssq
        r1[1, :MS] = 1.0
        r1[0, MS:] = 1.0
        r1[1, MS:] = tsq
        in_maps.append({
            "sT8": sT8,
            "tT8": tT8,
            "ssq": np.ascontiguousarray(ssq.reshape(MT, 128).T),
            "r1": r1,
            "tsq": np.ascontiguousarray(tsq.reshape(1, NS)),
        })
    return in_maps


def _run(s: np.ndarray, t: np.ndarray, trace: bool = False, tmpdir=None):
    if "nc" not in _CACHE:
        _CACHE["nc"] = _build()
    nc = _CACHE["nc"]
    in_maps = _prep_in_maps(s, t)
    res = run_bass_kernel_spmd(
        nc, in_maps, core_ids=list(range(8)), trace=trace, tmpdir=tmpdir
    )
    out = np.empty((N_S, N_T), dtype=np.float32)
    for c in range(8):
        si, tj = c // TB, c % TB
        out[si * MS:(si + 1) * MS, tj * NS:(tj + 1) * NS] = (
            res.results[c]["out"].astype(np.float32)
        )
    return out, res


def kernel(s: np.ndarray, t: np.ndarray) -> np.ndarray:
    s = np.ascontiguousarray(np.asarray(s, dtype=np.float32))
    t = np.ascontiguousarray(np.asarray(t, dtype=np.float32))
    assert s.shape == (N_S, D) and t.shape == (N_T, D)
    out, _ = _run(s, t)
    return out
